# revision 1
# baseline (speedup 1.0000x reference)
"""AQLM-quantized linear + LoRA for Trainium2, tensor-parallel over 8 NeuronCores.

Contract: kernel(**inputs) takes the FULL unsharded inputs of
nn_AQLMQuantizedLoRA (x, codes, codebooks, scales, bias, lora_A, lora_B)
and returns the full [4, 2048, 4096] float32 output.

Sharding (column-parallel): out_features (4096) split 8 ways; each core
computes out[:, :, c*512:(c+1)*512] = x @ W_eff[c]^T + bias[c] where
W_eff = dequant(codes)*scales + 2*(lora_B@lora_A).  x is replicated
(bf16), weights are shipped pre-transposed per core as bf16 k-tiles.
The device kernel streams x through the DMA-transpose XBAR into
[feature, token] tiles and runs 2048 bf16 matmuls per core (K=4096
contraction in 32 PSUM-accumulated steps), adds bias on the vector
engine, and writes the per-core [8192, 512] f32 slice.
"""
import sys
import numpy as np

sys.path.insert(0, '/opt/trn_rl_repo')

import ml_dtypes
import concourse.bass as bass
import concourse.mybir as mybir
from concourse import bacc
from concourse.tile import TileContext

F32 = mybir.dt.float32
BF16 = mybir.dt.bfloat16

N_CORES = 8
B, S = 4, 2048
B_TOK = B * S            # 8192
K_FEAT = 4096
NK = K_FEAT // 128       # 32
N_PER_CORE = 4096 // N_CORES  # 512
LORA_SCALING = 2.0
TOK_CHUNK = 512


def _build_kernel(nc):
    XB = nc.dram_tensor("XB", [B_TOK, K_FEAT], BF16, kind="ExternalInput")
    WT = nc.dram_tensor("WT", [NK, 128, N_PER_CORE], BF16, kind="ExternalInput")
    BIASR = nc.dram_tensor("BIASR", [128, N_PER_CORE], F32, kind="ExternalInput")
    OUT = nc.dram_tensor("OUT", [B_TOK, N_PER_CORE], F32, kind="ExternalOutput")

    n_chunks = B_TOK // TOK_CHUNK
    tpc = TOK_CHUNK // 128

    with TileContext(nc) as tc:
        with tc.tile_pool(name="persist", bufs=1) as pp, \
             tc.tile_pool(name="work", bufs=1) as wp, \
             tc.tile_pool(name="psum", bufs=1, space="PSUM") as psp:
            biasr = pp.tile([128, N_PER_CORE], F32, tag="biasr")
            nc.scalar.dma_start(biasr[:], BIASR.ap())
            wt = []
            for kt in range(NK):
                w_ = pp.tile([128, N_PER_CORE], BF16, tag=f"wt{kt}", name=f"wt{kt}")
                nc.scalar.dma_start(w_[:], WT[kt])
                wt.append(w_)

            for cc in range(n_chunks):
                t0 = cc * TOK_CHUNK
                slab = wp.tile([128, NK * TOK_CHUNK], BF16, tag="slab",
                               name=f"slab{cc}", bufs=2)
                nc.sync.dma_start(
                    slab[:].rearrange("p (k t) -> p k t", k=NK),
                    XB.ap()[t0:t0 + TOK_CHUNK, :], transpose=True)

                for tt in range(tpc):
                    ps = psp.tile([128, N_PER_CORE], F32,
                                  tag=f"ps{(cc * tpc + tt) % 4}",
                                  name=f"ps{cc}_{tt}")
                    for k in range(NK):
                        off = k * TOK_CHUNK + tt * 128
                        nc.tensor.matmul(ps[:], slab[:, off:off + 128], wt[k][:],
                                         start=(k == 0), stop=(k == NK - 1))
                    ot_sb = wp.tile([128, N_PER_CORE], F32, tag="ot",
                                    name=f"ot{cc}_{tt}", bufs=4)
                    nc.vector.tensor_add(ot_sb[:], ps[:], biasr[:])
                    nc.scalar.dma_start(
                        OUT.ap()[t0 + 128 * tt:t0 + 128 * (tt + 1), :], ot_sb[:])
    return nc


def _dequant_host(codes, codebooks, scales, lora_A, lora_B):
    cb = np.asarray(codebooks, np.float32).reshape(2, 256, 8)
    codes = np.asarray(codes)
    g = cb[0][codes[:, :, 0]] + cb[1][codes[:, :, 1]]      # [O, G, 8]
    w = g * np.asarray(scales, np.float32).reshape(-1, 1, 1)
    w = w.reshape(4096, 4096)
    return w + LORA_SCALING * (
        np.asarray(lora_B, np.float32) @ np.asarray(lora_A, np.float32))


def _host_prep(x, codes, codebooks, scales, bias, lora_A, lora_B, core, w_eff):
    o0 = core * N_PER_CORE
    wslice = w_eff[o0:o0 + N_PER_CORE]                     # [512 o, 4096 k]
    wt = np.ascontiguousarray(
        wslice.T.reshape(NK, 128, N_PER_CORE)).astype(ml_dtypes.bfloat16)
    biasr = np.broadcast_to(
        np.asarray(bias, np.float32)[o0:o0 + N_PER_CORE][None, :],
        (128, N_PER_CORE)).copy()
    return {"WT": wt, "BIASR": np.asarray(biasr, np.float32)}


_CACHE = {}


def _get_runner():
    if "runner" in _CACHE:
        return _CACHE["runner"]
    import jax
    from jax.sharding import Mesh, PartitionSpec
    from jax.experimental.shard_map import shard_map
    from concourse.bass2jax import (_bass_exec_p, partition_id_tensor,
                                    install_neuronx_cc_hook)

    nc = bacc.Bacc("TRN2", debug=False, num_devices=N_CORES)
    _build_kernel(nc)
    nc.compile()
    install_neuronx_cc_hook()

    partition_name = nc.partition_id_tensor.name if nc.partition_id_tensor else None
    in_names, out_names, out_avals, zero_outs = [], [], [], []
    for alloc in nc.m.functions[0].allocations:
        if not isinstance(alloc, mybir.MemoryLocationSet):
            continue
        name = alloc.memorylocations[0].name
        if alloc.kind == "ExternalInput":
            if name != partition_name:
                in_names.append(name)
        elif alloc.kind == "ExternalOutput":
            out_names.append(name)
            shape = tuple(alloc.tensor_shape)
            dtype = mybir.dt.np(alloc.dtype)
            out_avals.append(jax.core.ShapedArray(shape, dtype))
            zero_outs.append(np.zeros(shape, dtype))
    n_params = len(in_names)
    n_outs = len(out_avals)
    in_names_all = list(in_names) + out_names
    if partition_name is not None:
        in_names_all.append(partition_name)
    donate = tuple(range(n_params, n_params + n_outs))

    def _body(*args):
        operands = list(args)
        if partition_name is not None:
            operands.append(partition_id_tensor())
        outs = _bass_exec_p.bind(
            *operands,
            out_avals=tuple(out_avals),
            in_names=tuple(in_names_all),
            out_names=tuple(out_names),
            lowering_input_output_aliases=(),
            sim_require_finite=True,
            sim_require_nnan=True,
            nc=nc,
        )
        return tuple(outs)

    devices = jax.devices()[:N_CORES]
    mesh = Mesh(np.asarray(devices), ("core",))
    in_specs = (PartitionSpec("core"),) * (n_params + n_outs)
    out_specs = (PartitionSpec("core"),) * len(out_names)
    sharded = jax.jit(
        shard_map(_body, mesh=mesh, in_specs=in_specs, out_specs=out_specs,
                  check_rep=False),
        donate_argnums=donate, keep_unused=True)
    sharding = jax.sharding.NamedSharding(mesh, PartitionSpec("core"))

    runner = {
        "jax": jax, "sharded": sharded, "sharding": sharding,
        "in_names": in_names, "out_names": out_names,
        "out_avals": out_avals, "zero_outs": zero_outs,
    }
    _CACHE["runner"] = runner
    return runner


def run_device(in_maps):
    r = _get_runner()
    jax = r["jax"]
    concat_in = [
        jax.device_put(
            np.concatenate([np.asarray(in_maps[c][nm])
                            for c in range(N_CORES)], axis=0), r["sharding"])
        for nm in r["in_names"]
    ]
    zeros = [
        jax.device_put(
            np.zeros((N_CORES * z.shape[0], *z.shape[1:]), z.dtype),
            r["sharding"])
        for z in r["zero_outs"]
    ]
    out_arrs = r["sharded"](*concat_in, *zeros)
    for o in out_arrs:
        o.block_until_ready()
    return [
        {nm: np.asarray(out_arrs[i]).reshape(
            N_CORES, *r["out_avals"][i].shape)[c]
         for i, nm in enumerate(r["out_names"])}
        for c in range(N_CORES)
    ]


def kernel(x, codes, codebooks, scales, bias, lora_A, lora_B):
    x = np.asarray(x)
    w_eff = _dequant_host(codes, codebooks, scales, lora_A, lora_B)
    xb = np.ascontiguousarray(
        np.asarray(x, np.float32).reshape(B_TOK, K_FEAT)).astype(ml_dtypes.bfloat16)
    in_maps = []
    for c in range(N_CORES):
        m = _host_prep(x, codes, codebooks, scales, bias, lora_A, lora_B, c, w_eff)
        m["XB"] = xb
        in_maps.append(m)
    results = run_device(in_maps)
    out = np.concatenate([results[c]["OUT"] for c in range(N_CORES)], axis=1)
    return np.ascontiguousarray(out.reshape(B, S, 4096).astype(np.float32))


# revision 2
# speedup vs baseline: 165.5568x; 165.5568x over previous
"""AQLM-quantized linear + LoRA for Trainium2, tensor-parallel over 8 NeuronCores.

Contract: kernel(**inputs) takes the FULL unsharded inputs of
nn_AQLMQuantizedLoRA (x, codes, codebooks, scales, bias, lora_A, lora_B)
and returns the full [4, 2048, 4096] float32 output.

Sharding (column-parallel): out_features (4096) split 8 ways; each core
computes out[:, :, c*512:(c+1)*512] = x @ W_eff[c]^T + bias[c] where
W_eff = dequant(codes)*scales + 2*(lora_B@lora_A).  x is replicated
(bf16), weights are shipped pre-transposed per core as bf16 k-tiles.
The device kernel streams x through the DMA-transpose XBAR into
[feature, token] tiles and runs 2048 bf16 matmuls per core (K=4096
contraction in 32 PSUM-accumulated steps), adds bias on the vector
engine, and writes the per-core [8192, 512] f32 slice.
"""
import sys
import numpy as np

sys.path.insert(0, '/opt/trn_rl_repo')

import ml_dtypes
import concourse.bass as bass
import concourse.mybir as mybir
from concourse import bacc
from concourse.tile import TileContext

F32 = mybir.dt.float32
BF16 = mybir.dt.bfloat16

N_CORES = 8
B, S = 4, 2048
B_TOK = B * S            # 8192
K_FEAT = 4096
NK = K_FEAT // 128       # 32
N_PER_CORE = 4096 // N_CORES  # 512
LORA_SCALING = 2.0
TOK_CHUNK = 512


def _build_kernel(nc):
    XB = nc.dram_tensor("XB", [B_TOK, K_FEAT], BF16, kind="ExternalInput")
    WT = nc.dram_tensor("WT", [NK, 128, N_PER_CORE], BF16, kind="ExternalInput")
    BIASR = nc.dram_tensor("BIASR", [128, N_PER_CORE], F32, kind="ExternalInput")
    OUT = nc.dram_tensor("OUT", [B_TOK, N_PER_CORE], F32, kind="ExternalOutput")

    n_chunks = B_TOK // TOK_CHUNK
    tpc = TOK_CHUNK // 128

    with TileContext(nc) as tc:
        with tc.tile_pool(name="persist", bufs=1) as pp, \
             tc.tile_pool(name="work", bufs=1) as wp, \
             tc.tile_pool(name="psum", bufs=1, space="PSUM") as psp:
            biasr = pp.tile([128, N_PER_CORE], F32, tag="biasr")
            nc.scalar.dma_start(biasr[:], BIASR.ap())
            wt = []
            for kt in range(NK):
                w_ = pp.tile([128, N_PER_CORE], BF16, tag=f"wt{kt}", name=f"wt{kt}")
                nc.scalar.dma_start(w_[:], WT[kt])
                wt.append(w_)

            for cc in range(n_chunks):
                t0 = cc * TOK_CHUNK
                slab = wp.tile([128, NK * TOK_CHUNK], BF16, tag="slab",
                               name=f"slab{cc}", bufs=2)
                nc.sync.dma_start(
                    slab[:].rearrange("p (k t) -> p k t", k=NK),
                    XB.ap()[t0:t0 + TOK_CHUNK, :], transpose=True)

                for tt in range(tpc):
                    ps = psp.tile([128, N_PER_CORE], F32,
                                  tag=f"ps{(cc * tpc + tt) % 4}",
                                  name=f"ps{cc}_{tt}")
                    for k in range(NK):
                        off = k * TOK_CHUNK + tt * 128
                        nc.tensor.matmul(ps[:], slab[:, off:off + 128], wt[k][:],
                                         start=(k == 0), stop=(k == NK - 1))
                    ot_sb = wp.tile([128, N_PER_CORE], F32, tag="ot",
                                    name=f"ot{cc}_{tt}", bufs=4)
                    nc.vector.tensor_add(ot_sb[:], ps[:], biasr[:])
                    nc.scalar.dma_start(
                        OUT.ap()[t0 + 128 * tt:t0 + 128 * (tt + 1), :], ot_sb[:])
    return nc


def _dequant_host(codes, codebooks, scales, lora_A, lora_B):
    cb = np.asarray(codebooks, np.float32).reshape(2, 256, 8)
    codes = np.asarray(codes)
    g = cb[0][codes[:, :, 0]] + cb[1][codes[:, :, 1]]      # [O, G, 8]
    w = g * np.asarray(scales, np.float32).reshape(-1, 1, 1)
    w = w.reshape(4096, 4096)
    return w + LORA_SCALING * (
        np.asarray(lora_B, np.float32) @ np.asarray(lora_A, np.float32))


def _host_prep(x, codes, codebooks, scales, bias, lora_A, lora_B, core, w_eff):
    o0 = core * N_PER_CORE
    wslice = w_eff[o0:o0 + N_PER_CORE]                     # [512 o, 4096 k]
    wt = np.ascontiguousarray(
        wslice.T.reshape(NK, 128, N_PER_CORE)).astype(ml_dtypes.bfloat16)
    biasr = np.broadcast_to(
        np.asarray(bias, np.float32)[o0:o0 + N_PER_CORE][None, :],
        (128, N_PER_CORE)).copy()
    return {"WT": wt, "BIASR": np.asarray(biasr, np.float32)}


_CACHE = {}


def _get_runner():
    if "runner" in _CACHE:
        return _CACHE["runner"]
    import jax
    from jax.sharding import Mesh, PartitionSpec
    from jax.experimental.shard_map import shard_map
    from concourse.bass2jax import (_bass_exec_p, partition_id_tensor,
                                    install_neuronx_cc_hook)

    nc = bacc.Bacc("TRN2", debug=False, num_devices=N_CORES)
    _build_kernel(nc)
    nc.compile()
    install_neuronx_cc_hook()

    partition_name = nc.partition_id_tensor.name if nc.partition_id_tensor else None
    in_names, out_names, out_avals, zero_outs = [], [], [], []
    for alloc in nc.m.functions[0].allocations:
        if not isinstance(alloc, mybir.MemoryLocationSet):
            continue
        name = alloc.memorylocations[0].name
        if alloc.kind == "ExternalInput":
            if name != partition_name:
                in_names.append(name)
        elif alloc.kind == "ExternalOutput":
            out_names.append(name)
            shape = tuple(alloc.tensor_shape)
            dtype = mybir.dt.np(alloc.dtype)
            out_avals.append(jax.core.ShapedArray(shape, dtype))
            zero_outs.append(np.zeros(shape, dtype))
    n_params = len(in_names)
    n_outs = len(out_avals)
    in_names_all = list(in_names) + out_names
    if partition_name is not None:
        in_names_all.append(partition_name)
    donate = tuple(range(n_params, n_params + n_outs))

    def _body(*args):
        operands = list(args)
        if partition_name is not None:
            operands.append(partition_id_tensor())
        outs = _bass_exec_p.bind(
            *operands,
            out_avals=tuple(out_avals),
            in_names=tuple(in_names_all),
            out_names=tuple(out_names),
            lowering_input_output_aliases=(),
            sim_require_finite=True,
            sim_require_nnan=True,
            nc=nc,
        )
        return tuple(outs)

    devices = jax.devices()[:N_CORES]
    mesh = Mesh(np.asarray(devices), ("core",))
    in_specs = (PartitionSpec("core"),) * (n_params + n_outs)
    out_specs = (PartitionSpec("core"),) * len(out_names)
    sharded = jax.jit(
        shard_map(_body, mesh=mesh, in_specs=in_specs, out_specs=out_specs,
                  check_rep=False),
        donate_argnums=donate, keep_unused=True)
    sharding = jax.sharding.NamedSharding(mesh, PartitionSpec("core"))

    runner = {
        "jax": jax, "sharded": sharded, "sharding": sharding,
        "in_names": in_names, "out_names": out_names,
        "out_avals": out_avals, "zero_outs": zero_outs,
    }
    _CACHE["runner"] = runner
    return runner


def put_inputs(in_maps):
    r = _get_runner()
    jax = r["jax"]
    return [
        jax.device_put(
            np.concatenate([np.asarray(in_maps[c][nm])
                            for c in range(N_CORES)], axis=0), r["sharding"])
        for nm in r["in_names"]
    ]


def _make_zeros():
    """Zero-filled donated output buffers created on device (no upload)."""
    r = _get_runner()
    jax = r["jax"]
    import jax.numpy as jnp
    if "zeros_fn" not in r:
        shapes = [(N_CORES * z.shape[0], *z.shape[1:]) for z in r["zero_outs"]]
        dtypes = [z.dtype for z in r["zero_outs"]]

        def mk():
            return tuple(jnp.zeros(s, d) for s, d in zip(shapes, dtypes))

        r["zeros_fn"] = jax.jit(
            mk, out_shardings=tuple([r["sharding"]] * len(shapes)))
    zs = r["zeros_fn"]()
    for z in zs:
        z.block_until_ready()
    return list(zs)


def run_device_arrays(concat_in):
    r = _get_runner()
    zeros = _make_zeros()
    out_arrs = r["sharded"](*concat_in, *zeros)
    for o in out_arrs:
        o.block_until_ready()
    return out_arrs


def run_device(in_maps):
    r = _get_runner()
    out_arrs = run_device_arrays(put_inputs(in_maps))
    return [
        {nm: np.asarray(out_arrs[i]).reshape(
            N_CORES, *r["out_avals"][i].shape)[c]
         for i, nm in enumerate(r["out_names"])}
        for c in range(N_CORES)
    ]


def kernel(x, codes, codebooks, scales, bias, lora_A, lora_B):
    x = np.asarray(x)
    w_eff = _dequant_host(codes, codebooks, scales, lora_A, lora_B)
    xb = np.ascontiguousarray(
        np.asarray(x, np.float32).reshape(B_TOK, K_FEAT)).astype(ml_dtypes.bfloat16)
    in_maps = []
    for c in range(N_CORES):
        m = _host_prep(x, codes, codebooks, scales, bias, lora_A, lora_B, c, w_eff)
        m["XB"] = xb
        in_maps.append(m)
    results = run_device(in_maps)
    out = np.concatenate([results[c]["OUT"] for c in range(N_CORES)], axis=1)
    return np.ascontiguousarray(out.reshape(B, S, 4096).astype(np.float32))
